# revision 34
# baseline (speedup 1.0000x reference)
"""Trainium2 Bass kernel for nn_Loss5 (topk_masking).

reference:
    s_topk = top_k(x, 6)[0][:, 5]            # 6th largest per row, [B]
    s_y    = x[arange(B), y]                 # label score, [B]
    out    = mean(relu(1 + s_topk[None,:] - s_y[:,None]))   # over [B,B]
    element (i, j) of the [B,B] matrix = relu((1 - s_y[i]) + s_topk[j])

Strategy (8 cores, data-parallel over rows). Flipped-tail layout: the
[B,B] mean is partitioned over j (each core's LOCAL rows supply t_j =
s_topk[j] as the ACT per-partition bias) and the free dim runs over all
i via the allgathered bias vector b_i = 1 - s_y[i]:

  per core partial[j] = sum_i relu(b_i + t_j)

b depends only on the label gather (NOT on the top-k scan), so the
allgather + broadcast of b happen at the START, fully hidden behind
the 103 MB x-stream. Each row-group's loss ACT fires as soon as that
group's top-8 lands, overlapping stage 1. The only work after the
last x byte arrives: one small-chunk Max8, the final Max8, a split
ACT/DVE loss pass over [128, 4096], and the tiny partial store
(~7-9 us vs ~52 us for the gather-topk-at-the-end layout).

  - stage 1: exact per-row top-8 via DVE Max8 over DESCENDING column
    chunks (HW Max8 runs at ~0.70 ns/col vs DMA ~1.15 ns/col, so the
    DVE never backlogs and the 512-wide last chunk bounds the drain
    to ~0.5 us), then a final Max8 over per-chunk candidates.
  - stage 0 (hidden): indirect-DMA gather of s_y, bias = 1 - s_y,
    AllGather bias [512]->[4096] (2 KB payload), then replication to
    [128, 4096] PSUM via PE matmul ones[1,128].T @ bias[1,4096] --
    off the DMA fabric, so the x-stream keeps every AXI port.
  - stage 2 (mostly hidden): per-group ACT relu accumulation reading
    the bias tile from PSUM; the last group is split ACT (2432 cols,
    1-pass w/ accum) || DVE (1664 cols, relu then reduce) to halve
    the serial tail.
  - host: sum 8x[128,5] partials, divide by B^2.

Measured on HW (slope method, 8 cores): stage-1 x-stream ~225-232 us
(~443 GB/s/core, at the SBUF AXI fabric ceiling); DVE Max8 fully
hidden (dma-only probe == full slope). Raw bass blocks (not Tile):
with explicit semaphores every DMA carries at most one wait and the
x-stream queue (SP ring) carries nothing but the 32 x-chunk loads;
all small DMAs ride the ACT ring.
"""

import sys

import numpy as np

if "/opt/trn_rl_repo" not in sys.path:
    sys.path.insert(0, "/opt/trn_rl_repo")

import concourse.bass as bass
import concourse.mybir as mybir
from concourse.bass_utils import run_bass_kernel_spmd

B = 4096
V = 50257
NCORES = 8
RPC = B // NCORES          # rows per core = 512
G = RPC // 128             # row groups of 128 partitions = 4
K = 5                      # s_topk = (K+1)-th largest = top8[:, 5]

# Descending column chunks (max 16384 for Max8). HW Max8 (~0.70 ns/col)
# outruns the DMA stream (~1.15 ns/col) at every step of this schedule,
# so the DVE never backlogs; the 512-wide last chunk bounds the
# post-last-DMA Max8 drain to ~0.5 us.
_CHUNK_WS = [11468, 11036, 9233, 7180, 5294, 3562, 1972, 512]
assert sum(_CHUNK_WS) == V
_CHUNKS = []
_c0 = 0
for _w in _CHUNK_WS:
    _CHUNKS.append((_c0, _w))
    _c0 += _w
_NCHUNK = len(_CHUNK_WS)
NC8 = 8 * _NCHUNK          # candidate slots per group
W0 = _CHUNK_WS[0]

NSLOT = 3                  # x-tile load slots
ACT_COLS = 2432            # ACT share of the last group's loss row
                           # (ACT 1-pass ~0.95 ns/col vs DVE 2-pass ~1.4)

_NC_CACHE = {}


def _build_nc(repeat: int = 1, dve_min: bool = False, dma_once: bool = False):
    f32 = mybir.dt.float32
    i32 = mybir.dt.int32

    nc = bass.Bass()
    x = nc.declare_dram_parameter("x", [RPC, V], f32, isOutput=False)
    syoff = nc.declare_dram_parameter("syoff", [128, G], i32, isOutput=False)
    partial = nc.declare_dram_parameter("partial", [128, G + 1], f32, isOutput=True)

    sy_d = nc.dram_tensor("sy_cc_in", [RPC], f32)
    syfull_d = nc.dram_tensor("syfull_cc_out", [B], f32, addr_space="Shared")

    x_flat = x.ap().rearrange("a b -> (a b)")[:, None]

    from contextlib import ExitStack

    with ExitStack() as ctx:
        slots = ctx.enter_context(nc.sbuf_tensor("slots", [128, NSLOT * W0], f32))
        cand = ctx.enter_context(nc.sbuf_tensor("cand", [128, G * NC8], f32))
        top8 = ctx.enter_context(nc.sbuf_tensor("top8", [128, G * 8], f32))
        sy_sb = ctx.enter_context(nc.sbuf_tensor("sy", [128, G], f32))
        bias_sb = ctx.enter_context(nc.sbuf_tensor("bias", [128, G], f32))
        acc_sb = ctx.enter_context(nc.sbuf_tensor("acc", [128, G + 1], f32))
        so_sb = ctx.enter_context(nc.sbuf_tensor("so", [128, G], i32))
        syrow = ctx.enter_context(nc.sbuf_tensor("syrow", [1, B], f32))
        ones_sb = ctx.enter_context(nc.sbuf_tensor("ones", [1, 128], f32))
        tb = ctx.enter_context(nc.psum_tensor("tbp", [128, B], f32))
        scratch = ctx.enter_context(nc.sbuf_tensor("scratch", [128, B], f32))
        scratch2 = ctx.enter_context(nc.sbuf_tensor("scratch2", [128, B - ACT_COLS], f32))
        warm = ctx.enter_context(nc.sbuf_tensor("warm", [128, 8], f32))
        ld0 = ctx.enter_context(nc.semaphore("ld0"))
        ld1 = ctx.enter_context(nc.semaphore("ld1"))
        ld2 = ctx.enter_context(nc.semaphore("ld2"))
        mx = ctx.enter_context(nc.semaphore("mx"))
        fmx = ctx.enter_context(nc.semaphore("fmx"))
        so_s = ctx.enter_context(nc.semaphore("so_s"))
        gat = ctx.enter_context(nc.semaphore("gat"))
        bias_s = ctx.enter_context(nc.semaphore("bias_s"))
        syd_s = ctx.enter_context(nc.semaphore("syd_s"))
        cc = ctx.enter_context(nc.semaphore("cc"))
        syrow_s = ctx.enter_context(nc.semaphore("syrow_s"))
        ones_s = ctx.enter_context(nc.semaphore("ones_s"))
        tbb_s = ctx.enter_context(nc.semaphore("tbb_s"))
        outs = ctx.enter_context(nc.semaphore("outs"))
        act_s = ctx.enter_context(nc.semaphore("act_s"))
        accv_s = ctx.enter_context(nc.semaphore("accv_s"))
        warm_s = ctx.enter_context(nc.semaphore("warm_s"))
        block = ctx.enter_context(nc.Block())
        ld_sems = [ld0, ld1, ld2][:NSLOT]

        @block.sync
        def _(sync):
            # nothing but the x-chunk stream on this ring
            k = 0
            for rep in range(1 if dma_once else repeat):
                for g in range(G):
                    for j, (c0, w) in enumerate(_CHUNKS):
                        if k >= NSLOT:
                            # reader of this slot's previous contents done
                            sync.wait_ge(mx, k - NSLOT + 1)
                        s = (k % NSLOT) * W0
                        cs = (c0 + rep * 1237) % (V - w) if rep else c0
                        sync.dma_start(
                            out=slots[:, s : s + w],
                            in_=x[g * 128 : (g + 1) * 128, cs : cs + w],
                        ).then_inc(ld_sems[k % NSLOT], 16)
                        k += 1

        @block.vector
        def _(vector):
            vector.wait_ge(gat, 16 * G)
            nc.vector.tensor_scalar(
                out=bias_sb[:],
                in0=sy_sb[:],
                scalar1=-1.0,
                scalar2=1.0,
                op0=mybir.AluOpType.mult,
                op1=mybir.AluOpType.add,
            ).then_inc(bias_s, 1)
            k = 0
            for rep in range(repeat):
                for g in range(G):
                    for j, (c0, w) in enumerate(_CHUNKS):
                        s = (k % NSLOT) * W0
                        if not (dma_once and rep > 0):
                            vector.wait_ge(
                                ld_sems[k % NSLOT], 16 * (k // NSLOT + 1)
                            )
                        nc.vector.max(
                            cand[:, NC8 * g + 8 * j : NC8 * g + 8 * j + 8],
                            slots[:, s : s + (8 if dve_min else w)],
                        ).then_inc(mx, 1)
                        k += 1
                    vector.wait_ge(mx, _NCHUNK * (rep * G + g + 1))
                    nc.vector.max(
                        top8[:, 8 * g : 8 * g + 8], cand[:, NC8 * g : NC8 * (g + 1)]
                    ).then_inc(fmx, 1)
            # tail split: DVE covers the back of the LAST group's loss
            # row while ACT covers the front, splitting the serial tail
            # by the two engines' rates. Two passes (relu, then reduce):
            # the fused tensor_scalar accum_out silently drops the
            # accumulation in the neuronxcc lowering.
            vector.wait_ge(tbb_s, 8)
            vector.wait_ge(fmx, repeat * G)
            nc.vector.tensor_scalar(
                out=scratch2[:],
                in0=tb[:, ACT_COLS:],
                scalar1=top8[:, 8 * (G - 1) + K : 8 * (G - 1) + K + 1],
                scalar2=0.0,
                op0=mybir.AluOpType.add,
                op1=mybir.AluOpType.max,
            ).then_inc(accv_s, 1)
            vector.wait_ge(accv_s, 1)
            nc.vector.tensor_reduce(
                out=acc_sb[:, G : G + 1],
                in_=scratch2[:],
                axis=mybir.AxisListType.X,
                op=mybir.AluOpType.add,
            ).then_inc(accv_s, 1)

        @block.gpsimd
        def _(gpsimd):
            gpsimd.memset(warm[:], 0.0).then_inc(warm_s, 1)
            gpsimd.memset(ones_sb[:], 1.0).then_inc(ones_s, 1)
            gpsimd.dma_start(out=so_sb[:], in_=syoff.ap()).then_inc(so_s, 16)
            gpsimd.wait_ge(so_s, 16)
            for g in range(G):
                gpsimd.indirect_dma_start(
                    out=sy_sb[:, g : g + 1],
                    out_offset=None,
                    in_=x_flat,
                    in_offset=bass.IndirectOffsetOnAxis(
                        ap=so_sb[:, g : g + 1], axis=0
                    ),
                ).then_inc(gat, 16)
            gpsimd.wait_ge(syd_s, 16 * G)
            gpsimd.collective_compute(
                "AllGather",
                mybir.AluOpType.bypass,
                replica_groups=[list(range(NCORES))],
                ins=[sy_d[:]],
                outs=[syfull_d[:]],
            ).then_inc(cc, 1)

        @block.tensor
        def _(tensor):
            # broadcast the allgathered bias row across all 128 partitions
            # via ones[1,128].T @ syrow[1,B] -> PSUM, one matmul per 2 KB
            # PSUM bank -- keeps the 2 MB replication off the DMA fabric.
            tensor.wait_ge(ones_s, 1)
            tensor.wait_ge(syrow_s, 16)
            bank = 512
            for i in range(B // bank):
                nc.tensor.matmul(
                    tb[:, i * bank : (i + 1) * bank],
                    ones_sb[:],
                    syrow[:, i * bank : (i + 1) * bank],
                ).then_inc(tbb_s, 1)

        @block.scalar
        def _(scalar):
            # warm the relu table while stage 0/1 run
            scalar.wait_ge(warm_s, 1)
            nc.scalar.activation(
                out=warm[:],
                in_=warm[:],
                func=mybir.ActivationFunctionType.Relu,
            )
            scalar.wait_ge(bias_s, 1)
            for g in range(G):
                scalar.dma_start(
                    out=sy_d[bass.ts(g, 128)], in_=bias_sb[:, g : g + 1]
                ).then_inc(syd_s, 16)
            scalar.wait_ge(cc, 1)
            scalar.dma_start(out=syrow[:], in_=syfull_d.ap()).then_inc(syrow_s, 16)
            scalar.wait_ge(tbb_s, 8)
            for g in range(G):
                scalar.wait_ge(fmx, (repeat - 1) * G + g + 1)
                if g > 0:
                    scalar.wait_ge(act_s, g)
                last = g == G - 1
                nc.scalar.activation(
                    out=scratch[:, :ACT_COLS] if last else scratch[:],
                    in_=tb[:, :ACT_COLS] if last else tb[:],
                    func=mybir.ActivationFunctionType.Relu,
                    bias=top8[:, 8 * g + K : 8 * g + K + 1],
                    scale=1.0,
                    accum_out=acc_sb[:, g : g + 1],
                ).then_inc(act_s, 1)
            scalar.wait_ge(act_s, G)
            scalar.wait_ge(accv_s, 2)
            scalar.dma_start(out=partial.ap(), in_=acc_sb[:]).then_inc(outs, 16)
            scalar.wait_ge(outs, 16)

    return nc


def _get_nc(repeat: int = 1, dve_min: bool = False, dma_once: bool = False):
    key = ("nc", repeat, dve_min, dma_once)
    if key not in _NC_CACHE:
        _NC_CACHE[key] = _build_nc(repeat, dve_min, dma_once)
    return _NC_CACHE[key]


def _run(x, y, trace=False):
    x = np.ascontiguousarray(np.asarray(x, dtype=np.float32))
    y = np.asarray(y).astype(np.int64).reshape(B)
    assert x.shape == (B, V)

    nc = _get_nc()
    in_maps = []
    r = np.arange(RPC, dtype=np.int64)
    for c in range(NCORES):
        rows = slice(c * RPC, (c + 1) * RPC)
        yl = y[rows]
        off = (r * V + yl).astype(np.int32).reshape(G, 128).T.copy()
        in_maps.append({"x": x[rows], "syoff": off})

    res = run_bass_kernel_spmd(nc, in_maps, list(range(NCORES)), trace=trace)
    total = 0.0
    for c in range(NCORES):
        total += float(res.results[c]["partial"].astype(np.float64).sum())
    out = np.array(total / (float(B) * float(B)), dtype=np.float32)
    return out, res


def kernel(x, y, k):
    assert int(k) == K
    out, _ = _run(x, y, trace=False)
    return out
